# revision 4
# baseline (speedup 1.0000x reference)
"""RWKV-4 style WKV attention (nn_Attention_4234837754291) on 8 TRN2 NeuronCores.

Self-contained Bass/Tile kernel. Sharding: core i -> (batch b = i//2,
D-half h = i%2). Each core runs the full pipeline for its (b, h): k/v/r
projections (contract full D, produce its DL=512 output channels), the
linear-space WKV scan over T on those channels, the sigmoid gate, and a
partial output projection through its DL rows of W_out.T. The host sums the
two D-half partial outputs per batch.

The time-mix inputs y_p = x*mix_p + last_x*(1-mix_p) are precomputed on the
host in a partition-major blocked layout so every per-chunk load is one
contiguous [128, 4096] DMA, and the device does only matmuls, activations,
the two scans and the wkv arithmetic:

  k = yk.T @ Wk.T (bf16), v = ... (bf16), r = ... (fp8e4 DoubleRow, 2x PE)
  ek = exp(k)                                                (f16)
  A_t = ew*A_{t-1} + ekv_t ;  B_t = ew*B_{t-1} + ek_t        (ew = exp(-exp(td)))
  nm = A_t + c*ekv_t ; dn = B_t + c*ek_t                     (c = ew*e^u - 1)
  out_t = (nm / (dn*(1+e^{-r_t}))) @ W_out.T[dsl]            (sigmoid folded
                                                              into the denom)

Key scheduling/precision decisions (measured on HW):
 - scan outputs A/B in f16 (bf16 quadruples the output error; f32 loses the
   16-bit DVE path). The scan carry is fp32 internally.
 - the sigmoid gate is folded into the denominator:
   wkv*sigmoid(r) = nm / (dn*(1+e^{-r})), which removes the Tanh, the +1.0
   add and one DVE multiply; er=exp(-r) is kept in f32 so it can't overflow.
 - nm and dn are produced by ONE 1024-wide scalar_tensor_tensor over the
   packed [ekv|ek] and [A|B] tiles (c as a f16 per-partition scalar);
   1024-wide STT measures 1279ns vs 2x745ns for two 512s.
 - the scan decay operand is a stride-0 broadcast AP from a [128,MB] tile
   (measured same speed as a full tensor, kills the 512KB ewm DMA).
 - fp8 is accuracy-viable ONLY for the r projection; k/v/out fp8 blow the
   2e-2 tolerance (numpy sim: out .054, v .042, k .021 vs budget .02).
 - a post-compile pass deletes an InstLdweights identical to the previous
   one on the PE queue (sync-free ones only): the two 512-wide halves of
   each out-projection matmul share their stationary tile.
 - the out-projection for segment s-1 is issued after the projections of
   segment s (software pipelining); the last chunk is split 2x256 to
   overlap the tail.
 - head: chunk-0 y loads + weights issue on the sync-queue DGE in priority
   order (yk0,wk,yv0,wv,...) since the gpsimd DGE only starts flowing at
   ~12us (library-load preamble); wo goes via the scalar DGE; warmup
   matmuls use a DVE-memset tile so they start at ~2us, not ~8us, keeping
   the PE HAM clock at full speed when the real matmuls arrive.
"""
import os
import numpy as np
import ml_dtypes
from contextlib import ExitStack

import concourse.bacc as bacc
import concourse.tile as tile
import concourse.mybir as mybir
from concourse.bass import AP
from concourse.bass_utils import run_bass_kernel_spmd

F32 = mybir.dt.float32
F16 = mybir.dt.float16
BF16 = mybir.dt.bfloat16
F8 = mybir.dt.float8e4
AF = mybir.ActivationFunctionType
OP = mybir.AluOpType

B, T, D = 4, 4096, 1024
DL = 512          # D-half per core
TC = 512          # time chunk
NCORES = 8

_NC_CACHE = {}


def _ap_sig(pap):
    return (str(pap.memref), pap.offset, tuple(map(tuple, pap.ap)), pap.dtype)


def _dedupe_ldweights(nc):
    """Delete an InstLdweights whose weights AP matches the previous
    InstLdweights in the same block's PE stream, when it carries no sync
    info. The PE array then just keeps the already-loaded weights."""
    removed = 0
    for f in nc.m.functions:
        for blk in f.blocks:
            insts = list(blk.instructions)
            last_sig = None
            keep = []
            for i in insts:
                nm = type(i).__name__
                if nm == 'InstLdweights':
                    sig = (_ap_sig(i.ins[0]), i.perf_mode, i.is_transpose,
                           i.tile_position, i.tile_size)
                    si = i.sync_info
                    clean = si is None or (len(si.on_wait) == 0
                                           and len(si.on_update) == 0)
                    if sig == last_sig and clean:
                        removed += 1
                        continue
                    last_sig = sig
                elif nm == 'InstMatmult':
                    pass          # matmuls don't invalidate loaded weights
                elif getattr(i, 'engine', None) == mybir.EngineType.PE:
                    last_sig = None  # drains/branches on PE: be conservative
                keep.append(i)
            if removed and len(keep) != len(insts):
                blk.instructions.clear()
                for i in keep:
                    blk.instructions.append(i)
    return removed


def _build(D_=D, DL_=DL, T_=T, TC_=TC, n_devices=NCORES):
    KB, MB, NCH = D_ // 128, DL_ // 128, T_ // TC_
    YW = KB * TC_          # per-chunk packed y width (4096)
    NP = KB // 2           # fp8 DoubleRow kb-pairs

    nc = bacc.Bacc("TRN2", target_bir_lowering=False, debug=False,
                   num_devices=n_devices)
    # all inputs pre-blocked on the host to [128, ...] partition-major
    yk = nc.dram_tensor("yk", (128, NCH * YW), BF16, kind="ExternalInput").ap()
    yv = nc.dram_tensor("yv", (128, NCH * YW), BF16, kind="ExternalInput").ap()
    yr = nc.dram_tensor("yr", (128, NCH * YW), F8, kind="ExternalInput").ap()
    wk = nc.dram_tensor("wk", (128, KB * DL_), BF16, kind="ExternalInput").ap()
    wv = nc.dram_tensor("wv", (128, KB * DL_), BF16, kind="ExternalInput").ap()
    wr = nc.dram_tensor("wr", (128, KB * DL_), F8, kind="ExternalInput").ap()
    wo = nc.dram_tensor("wo", (128, MB * D_), F16, kind="ExternalInput").ap()
    cvec = nc.dram_tensor("cvec", (128, MB), F16, kind="ExternalInput").ap()
    ewv = nc.dram_tensor("ewv", (128, MB), F16, kind="ExternalInput").ap()
    out = nc.dram_tensor("out", (T_, D_), F16, kind="ExternalOutput").ap()

    with tile.TileContext(nc) as tc, ExitStack() as ctx:
        wpool = ctx.enter_context(tc.tile_pool(name="weights", bufs=1))

        # PE warmup: the warm tile is memset on the DVE (ready ~2us; the
        # gpsimd DGE spends ~8us in its library-load preamble) so the dummy
        # matmuls keep the PE HAM clock at 2.4 GHz until real work arrives.
        warm = wpool.tile([128, 640], BF16, tag="warm")
        nc.vector.memset(warm[:], 0.0)

        # Head loads in kb-pair pieces, spread over three DGE queues so the
        # chunk-0 projections can start on piece 0 while later pieces are
        # still in flight (dep tracking is range-granular):
        #   sync DGE:   y0k/wk interleaved, then cvec/ew (k-phase critical)
        #   scalar DGE: y0v/wv interleaved, then wo
        #   gpsimd DGE: y0r/wr, then the chunk-1+ loads follow in the s-loop
        y0k = wpool.tile([128, YW], BF16, tag="y0k")
        wkall = wpool.tile([128, KB * DL_], BF16, tag="wk")
        for p in range(NP):
            nc.sync.dma_start(y0k[:, p * 2 * TC_:(p + 1) * 2 * TC_],
                              yk[:, p * 2 * TC_:(p + 1) * 2 * TC_])
            nc.sync.dma_start(wkall[:, p * 2 * DL_:(p + 1) * 2 * DL_],
                              wk[:, p * 2 * DL_:(p + 1) * 2 * DL_])
        cvec_sb = wpool.tile([128, MB], F16, tag="cvec")
        nc.sync.dma_start(cvec_sb[:], cvec[:])
        ew_dma = wpool.tile([128, MB], F16, tag="ew_dma")
        nc.sync.dma_start(ew_dma[:], ewv[:])
        # copy on the DVE so the stride-0 broadcast reads used by the scans
        # are ordered behind the write by same-engine program order
        ew_sb = wpool.tile([128, MB], F16, tag="ew_sb")
        nc.vector.tensor_copy(ew_sb[:], ew_dma[:])

        y0v = wpool.tile([128, YW], BF16, tag="y0v")
        wvall = wpool.tile([128, KB * DL_], BF16, tag="wv")
        for p in range(NP):
            nc.scalar.dma_start(y0v[:, p * 2 * TC_:(p + 1) * 2 * TC_],
                                yv[:, p * 2 * TC_:(p + 1) * 2 * TC_])
            nc.scalar.dma_start(wvall[:, p * 2 * DL_:(p + 1) * 2 * DL_],
                                wv[:, p * 2 * DL_:(p + 1) * 2 * DL_])
        woall = wpool.tile([128, MB * D_], F16, tag="wo")
        nc.scalar.dma_start(woall[:], wo[:])

        y0r = []
        wr_sb = []
        for p in range(NP):
            t = wpool.tile([128, 2, TC_], F8, tag=f"y0r{p}")
            nc.gpsimd.dma_start(t[:], yr[:, p * 2 * TC_:(p + 1) * 2 * TC_])
            y0r.append(t)
            t = wpool.tile([128, 2, DL_], F8, tag=f"wr{p}")
            nc.gpsimd.dma_start(t[:], wr[:, p * 2 * DL_:(p + 1) * 2 * DL_])
            wr_sb.append(t)

        y_pool = ctx.enter_context(tc.tile_pool(name="y", bufs=2))
        pp_pool = ctx.enter_context(tc.tile_pool(name="pp", bufs=4, space="PSUM"))
        po_pool = ctx.enter_context(tc.tile_pool(name="po", bufs=1, space="PSUM"))
        ee_pool = ctx.enter_context(tc.tile_pool(name="ee", bufs=2))
        er_pool = ctx.enter_context(tc.tile_pool(name="er", bufs=2))
        ab_pool = ctx.enter_context(tc.tile_pool(name="ab", bufs=2))
        nd_pool = ctx.enter_context(tc.tile_pool(name="nd", bufs=2))
        dn_pool = ctx.enter_context(tc.tile_pool(name="dn", bufs=2))
        vv_pool = ctx.enter_context(tc.tile_pool(name="vv", bufs=2))
        ws_pool = ctx.enter_context(tc.tile_pool(name="ws", bufs=2))
        ob_pool = ctx.enter_context(tc.tile_pool(name="ob", bufs=2))

        def ew_bc(mb, ln):
            base = ew_sb[:, mb:mb + 1]
            return AP(base.tensor, base.offset, [list(base.ap[0]), [0, ln]])

        prevA = [None] * MB
        prevB = [None] * MB
        # segments: (chunk, off, ln, load). The last chunk is split in half
        # so its out-projection overlaps the tail of the vector chain.
        segs = [(c, 0, TC_, True) for c in range(NCH - 1)]
        segs += [(NCH - 1, 0, TC_ // 2, True),
                 (NCH - 1, TC_ // 2, TC_ // 2, False)]
        wss_hist = {}
        ykt = yvt = yrt = None
        for s in range(len(segs) + 1):
            if s < len(segs):
                c, off, ln, load = segs[s]
                if load:
                    if c == 0:
                        ykt, yvt, yrt = y0k, y0v, y0r
                    else:
                        ykt = y_pool.tile([128, YW], BF16, tag="yk")
                        nc.gpsimd.dma_start(ykt[:], yk[:, c * YW:(c + 1) * YW])
                        yvt = y_pool.tile([128, YW], BF16, tag="yv")
                        nc.gpsimd.dma_start(yvt[:], yv[:, c * YW:(c + 1) * YW])
                        yrt = []
                        for p in range(NP):
                            yt = y_pool.tile([128, 2, TC_], F8, tag=f"yr{p}")
                            nc.gpsimd.dma_start(
                                yt[:],
                                yr[:, c * YW + p * 2 * TC_:
                                   c * YW + (p + 1) * 2 * TC_])
                            yrt.append(yt)

                wss = []
                ees, ers, vvs = [], [], []
                if s == 0:
                    # chunk 0: kb-outer phased projections so the first
                    # matmuls only need DMA piece 0, not the whole 1MB
                    # weight + 1MB y load (pieces land progressively).
                    kps = [pp_pool.tile([128, TC_], F32, tag="pp",
                                        name=f"kp0_{mb}") for mb in range(MB)]
                    for i in range(8):
                        nc.tensor.matmul(kps[0][:], warm[:, 0:128],
                                         warm[:, 128:640],
                                         start=True, stop=True)
                    for kb in range(KB):
                        for mb in range(MB):
                            nc.tensor.matmul(
                                kps[mb][:],
                                wkall[:, kb * DL_ + mb * 128:
                                      kb * DL_ + (mb + 1) * 128],
                                ykt[:, kb * TC_:kb * TC_ + TC_],
                                start=(kb == 0), stop=(kb == KB - 1))
                    for mb in range(MB):
                        ee = ee_pool.tile([128, 2 * TC_], F16, tag=f"ee{mb}",
                                          name=f"ee0_{mb}")
                        nc.scalar.activation(ee[:, TC_:2 * TC_], kps[mb][:],
                                             AF.Exp)
                        ees.append(ee)
                    vps = [pp_pool.tile([128, TC_], F32, tag="pp",
                                        name=f"vp0_{mb}") for mb in range(MB)]
                    for kb in range(KB):
                        for mb in range(MB):
                            nc.tensor.matmul(
                                vps[mb][:],
                                wvall[:, kb * DL_ + mb * 128:
                                      kb * DL_ + (mb + 1) * 128],
                                yvt[:, kb * TC_:kb * TC_ + TC_],
                                start=(kb == 0), stop=(kb == KB - 1))
                    for mb in range(MB):
                        vv = vv_pool.tile([128, TC_], F16, tag=f"vv{mb}",
                                          name=f"vv0_{mb}")
                        nc.scalar.copy(vv[:], vps[mb][:])
                        vvs.append(vv)
                    rps = [pp_pool.tile([128, TC_], F32, tag="pp",
                                        name=f"rp0_{mb}") for mb in range(MB)]
                    for p in range(NP):
                        for mb in range(MB):
                            nc.tensor.matmul(
                                rps[mb][:],
                                wr_sb[p][:, :, mb * 128:(mb + 1) * 128],
                                yrt[p][:, :, 0:TC_],
                                start=(p == 0), stop=(p == NP - 1),
                                perf_mode=mybir.MatmulPerfMode.DoubleRow)
                    for mb in range(MB):
                        er = er_pool.tile([128, TC_], F32, tag=f"er{mb}",
                                          name=f"er0_{mb}")
                        nc.scalar.activation(er[:], rps[mb][:], AF.Exp,
                                             scale=-1.0)
                        ers.append(er)
                for mb in range(MB):
                    if s == 0:
                        ee, er, vv = ees[mb], ers[mb], vvs[mb]
                    else:
                        kp = pp_pool.tile([128, TC_], F32, tag="pp")
                        for kb in range(KB):
                            nc.tensor.matmul(
                                kp[:, 0:ln],
                                wkall[:, kb * DL_ + mb * 128:
                                      kb * DL_ + (mb + 1) * 128],
                                ykt[:, kb * TC_ + off:kb * TC_ + off + ln],
                                start=(kb == 0), stop=(kb == KB - 1))
                        vp = pp_pool.tile([128, TC_], F32, tag="pp")
                        for kb in range(KB):
                            nc.tensor.matmul(
                                vp[:, 0:ln],
                                wvall[:, kb * DL_ + mb * 128:
                                      kb * DL_ + (mb + 1) * 128],
                                yvt[:, kb * TC_ + off:kb * TC_ + off + ln],
                                start=(kb == 0), stop=(kb == KB - 1))
                        rp = pp_pool.tile([128, TC_], F32, tag="pp")
                        for p in range(NP):
                            nc.tensor.matmul(
                                rp[:, 0:ln],
                                wr_sb[p][:, :, mb * 128:(mb + 1) * 128],
                                yrt[p][:, :, off:off + ln],
                                start=(p == 0), stop=(p == NP - 1),
                                perf_mode=mybir.MatmulPerfMode.DoubleRow)

                        # EE = [ekv | ek] f16
                        ee = ee_pool.tile([128, 2 * TC_], F16, tag=f"ee{mb}")
                        nc.scalar.activation(ee[:, TC_:TC_ + ln], kp[:, 0:ln],
                                             AF.Exp)
                        # er = exp(-r) f32 (f32 so large -r can't overflow)
                        er = er_pool.tile([128, TC_], F32, tag=f"er{mb}")
                        nc.scalar.activation(er[:, 0:ln], rp[:, 0:ln], AF.Exp,
                                             scale=-1.0)
                        # evict v via the scalar engine: frees the PSUM bank
                        # for the PE and makes the ekv multiply all-f16
                        vv = vv_pool.tile([128, TC_], F16, tag=f"vv{mb}")
                        nc.scalar.copy(vv[:, 0:ln], vp[:, 0:ln])
                    nc.vector.tensor_tensor(ee[:, 0:ln], ee[:, TC_:TC_ + ln],
                                            vv[:, 0:ln], OP.mult)

                    # A/B scan outputs in f16, decay as stride-0 broadcast
                    ab = ab_pool.tile([128, 2 * TC_], F16, tag=f"ab{mb}")
                    initA = 0.0 if s == 0 else prevA[mb]
                    nc.vector.tensor_tensor_scan(ab[:, 0:ln], ew_bc(mb, ln),
                                                 ee[:, 0:ln], initA,
                                                 OP.mult, OP.add)
                    initB = 0.0 if s == 0 else prevB[mb]
                    nc.vector.tensor_tensor_scan(ab[:, TC_:TC_ + ln],
                                                 ew_bc(mb, ln),
                                                 ee[:, TC_:TC_ + ln], initB,
                                                 OP.mult, OP.add)
                    prevA[mb] = ab[:, ln - 1:ln]
                    prevB[mb] = ab[:, TC_ + ln - 1:TC_ + ln]

                    # [nm | dn] = c*[ekv | ek] + [A | B] in ONE wide STT
                    cs = cvec_sb[:, mb:mb + 1]
                    nd = nd_pool.tile([128, 2 * TC_], F16, tag=f"nd{mb}")
                    if ln == TC_:
                        nc.vector.scalar_tensor_tensor(nd[:], ee[:], cs,
                                                       ab[:], OP.mult, OP.add)
                    else:
                        nc.vector.scalar_tensor_tensor(
                            nd[:, 0:ln], ee[:, 0:ln], cs, ab[:, 0:ln],
                            OP.mult, OP.add)
                        nc.vector.scalar_tensor_tensor(
                            nd[:, TC_:TC_ + ln], ee[:, TC_:TC_ + ln], cs,
                            ab[:, TC_:TC_ + ln], OP.mult, OP.add)

                    # dn' = (er + 1) * dn ; ws = nm / dn'
                    dnp = dn_pool.tile([128, TC_], F32, tag=f"dn{mb}")
                    nc.vector.scalar_tensor_tensor(dnp[:, 0:ln], er[:, 0:ln],
                                                   1.0, nd[:, TC_:TC_ + ln],
                                                   OP.add, OP.mult)
                    nc.vector.reciprocal_approx_fast(dnp[:, 0:ln],
                                                     dnp[:, 0:ln])
                    ws = ws_pool.tile([128, TC_], F16, tag=f"ws{mb}")
                    nc.vector.tensor_tensor(ws[:, 0:ln], nd[:, 0:ln],
                                            dnp[:, 0:ln], OP.mult)
                    wss.append(ws)
                wss_hist[s] = (wss, c * TC_ + off, ln)

            if s >= 1:
                # out-projection for segment s-1 (deferred so the PE never
                # waits on the vector chain of the same segment)
                wssp, tbase, lnp = wss_hist.pop(s - 1)
                for pair in range(lnp // 256):
                    pos = [po_pool.tile([128, D_], F32, tag=f"po{i}",
                                        name=f"po{i}") for i in range(2)]
                    for mb in range(MB):
                        for i, tb in enumerate((pair * 2, pair * 2 + 1)):
                            for half in range(2):
                                nc.tensor.matmul(
                                    pos[i][:, half * 512:(half + 1) * 512],
                                    wssp[mb][:, tb * 128:(tb + 1) * 128],
                                    woall[:, mb * D_ + half * 512:
                                          mb * D_ + (half + 1) * 512],
                                    start=(mb == 0), stop=(mb == MB - 1))
                    for i, tb in enumerate((pair * 2, pair * 2 + 1)):
                        ob = ob_pool.tile([128, D_], F16, tag="ob")
                        nc.scalar.copy(ob[:], pos[i][:])
                        nc.sync.dma_start(
                            out[tbase + tb * 128:tbase + (tb + 1) * 128, :],
                            ob[:])

    nc.compile()
    n = _dedupe_ldweights(nc)
    if os.environ.get("KERNEL_DEBUG"):
        print(f"dedupe_ldweights removed {n}")
    return nc


def get_nc():
    if "nc" not in _NC_CACHE:
        _NC_CACHE["nc"] = _build()
    return _NC_CACHE["nc"]


def _blk_y(arr, dt):
    # [D, T] -> [128, NCH*KB*TC]: y[kb*128+p, c*TC+t] -> [p, c*YW + kb*TC + t]
    KB, NCH = D // 128, T // TC
    a = arr.reshape(KB, 128, NCH, TC).transpose(1, 2, 0, 3)
    return np.ascontiguousarray(a.reshape(128, NCH * KB * TC)).astype(dt)


def _blk_yr(arr, dt):
    # [D, T] -> [128, NCH*KB*TC] with DoubleRow kb-pair packing:
    # yr[(2q+i)*128+p, c*TC+t] -> [p, c*YW + q*2*TC + i*TC + t]
    KB, NCH = D // 128, T // TC
    a = arr.reshape(KB // 2, 2, 128, NCH, TC).transpose(2, 3, 0, 1, 4)
    return np.ascontiguousarray(a.reshape(128, NCH * KB * TC)).astype(dt)


def _blk_w(arr, dt):
    # [D, DL] -> [128, KB*DL]: w[kb*128+p, j] -> [p, kb*DL + j]
    KB = D // 128
    a = arr.reshape(KB, 128, DL).transpose(1, 0, 2)
    return np.ascontiguousarray(a.reshape(128, KB * DL)).astype(dt)


def _blk_wr(arr, dt):
    # [D, DL] -> [128, KB*DL] with DoubleRow kb-pair packing
    KB = D // 128
    a = arr.reshape(KB // 2, 2, 128, DL).transpose(2, 0, 1, 3)
    return np.ascontiguousarray(a.reshape(128, KB * DL)).astype(dt)


def _blk_wo(arr, dt):
    # [DL, D] -> [128, MB*D]: wo[mb*128+p, i] -> [p, mb*D + i]
    MB = DL // 128
    a = arr.reshape(MB, 128, D).transpose(1, 0, 2)
    return np.ascontiguousarray(a.reshape(128, MB * D)).astype(dt)


def make_in_maps(x, time_decay, time_first, time_mix_k, time_mix_v, time_mix_r,
                 W_key, W_value, W_receptance, W_output):
    x = np.asarray(x, np.float32)
    time_decay = np.asarray(time_decay, np.float64)
    time_first = np.asarray(time_first, np.float64)
    mk = np.asarray(time_mix_k, np.float32).reshape(-1)
    mv = np.asarray(time_mix_v, np.float32).reshape(-1)
    mr = np.asarray(time_mix_r, np.float32).reshape(-1)
    W_key = np.asarray(W_key, np.float32)
    W_value = np.asarray(W_value, np.float32)
    W_receptance = np.asarray(W_receptance, np.float32)
    W_output = np.asarray(W_output, np.float32)

    MB = DL // 128
    ew = np.exp(-np.exp(time_decay))
    c = (ew * np.exp(time_first) - 1.0).astype(np.float32)
    ew = ew.astype(np.float32)

    def blocked(vec, nb, dt):
        return np.ascontiguousarray(vec.reshape(nb, 128).T.astype(dt))

    # host time-mix: y_p[b] = (x*m_p + last_x*(1-m_p)).T  as [D, T]
    last_x = np.concatenate([np.zeros((B, 1, D), np.float32), x[:, :-1, :]],
                            axis=1)
    ys = {}
    for nm, m in (("yk", mk), ("yv", mv), ("yr", mr)):
        ym = x * m + last_x * (1.0 - m)
        if nm == "yr":
            ys[nm] = [_blk_yr(ym[b].T, ml_dtypes.float8_e4m3)
                      for b in range(B)]
        else:
            ys[nm] = [_blk_y(ym[b].T, ml_dtypes.bfloat16) for b in range(B)]

    halves = []
    for h in range(2):
        dsl = slice(h * DL, (h + 1) * DL)
        halves.append({
            "wk": _blk_w(W_key.T[:, dsl], ml_dtypes.bfloat16),
            "wv": _blk_w(W_value.T[:, dsl], ml_dtypes.bfloat16),
            "wr": _blk_wr(W_receptance.T[:, dsl], ml_dtypes.float8_e4m3),
            "wo": _blk_wo(W_output.T[dsl, :], np.float16),
            "cvec": blocked(c[dsl], MB, np.float16),
            "ewv": blocked(ew[dsl], MB, np.float16),
        })

    in_maps = []
    for i in range(NCORES):
        b, h = i // 2, i % 2
        m = dict(halves[h])
        m["yk"] = ys["yk"][b]
        m["yv"] = ys["yv"][b]
        m["yr"] = ys["yr"][b]
        in_maps.append(m)
    return in_maps


def run(in_maps, trace=False):
    nc = get_nc()
    return run_bass_kernel_spmd(nc, in_maps, core_ids=list(range(NCORES)),
                                trace=trace)


def kernel(**inputs):
    in_maps = make_in_maps(**inputs)
    res = run(in_maps, trace=bool(int(os.environ.get("KERNEL_TRACE", "0"))))
    out = np.zeros((B, T, D), np.float32)
    for i in range(NCORES):
        out[i // 2] += res.results[i]["out"].astype(np.float32)
    if res.exec_time_ns is not None:
        print(f"HW exec time: {res.exec_time_ns} ns")
    return out


# revision 5
# speedup vs baseline: 1.0144x; 1.0144x over previous
"""RWKV-4 style WKV attention (nn_Attention_4234837754291) on 8 TRN2 NeuronCores.

Self-contained Bass/Tile kernel. Sharding: core i -> (batch b = i//2,
D-half h = i%2). Each core runs the full pipeline for its (b, h): k/v/r
projections (contract full D, produce its DL=512 output channels), the
linear-space WKV scan over T on those channels, the sigmoid gate, and a
partial output projection through its DL rows of W_out.T. The host sums the
two D-half partial outputs per batch.

The time-mix inputs y_p = x*mix_p + last_x*(1-mix_p) are precomputed on the
host in a partition-major blocked layout so every per-chunk load is one
contiguous [128, 4096] DMA, and the device does only matmuls, activations,
the two scans and the wkv arithmetic:

  k = yk.T @ Wk.T (bf16), v = ... (bf16), r = ... (fp8e4 DoubleRow, 2x PE)
  ek = exp(k)                                                (f16)
  A_t = ew*A_{t-1} + ekv_t ;  B_t = ew*B_{t-1} + ek_t        (ew = exp(-exp(td)))
  nm = A_t + c*ekv_t ; dn = B_t + c*ek_t                     (c = ew*e^u - 1)
  out_t = (nm / (dn*(1+e^{-r_t}))) @ W_out.T[dsl]            (sigmoid folded
                                                              into the denom)

Key scheduling/precision decisions (measured on HW):
 - scan outputs A/B in f16 (bf16 quadruples the output error; f32 loses the
   16-bit DVE path). The scan carry is fp32 internally.
 - the sigmoid gate is folded into the denominator:
   wkv*sigmoid(r) = nm / (dn*(1+e^{-r})), which removes the Tanh, the +1.0
   add and one DVE multiply; er=exp(-r) is kept in f32 so it can't overflow.
 - nm and dn are produced by ONE 1024-wide scalar_tensor_tensor over the
   packed [ekv|ek] and [A|B] tiles (c as a f16 per-partition scalar);
   1024-wide STT measures 1279ns vs 2x745ns for two 512s.
 - the scan decay operand is a stride-0 broadcast AP from a [128,MB] tile
   (measured same speed as a full tensor, kills the 512KB ewm DMA).
 - fp8 is accuracy-viable ONLY for the r projection; k/v/out fp8 blow the
   2e-2 tolerance (numpy sim: out .054, v .042, k .021 vs budget .02).
 - a post-compile pass deletes an InstLdweights identical to the previous
   one on the PE queue (sync-free ones only): the two 512-wide halves of
   each out-projection matmul share their stationary tile.
 - the out-projection for segment s-1 is issued after the projections of
   segment s (software pipelining); the last chunk is split 2x256 to
   overlap the tail.
 - head: chunk-0 y loads + weights issue on the sync-queue DGE in priority
   order (yk0,wk,yv0,wv,...) since the gpsimd DGE only starts flowing at
   ~12us (library-load preamble); wo goes via the scalar DGE; warmup
   matmuls use a DVE-memset tile so they start at ~2us, not ~8us, keeping
   the PE HAM clock at full speed when the real matmuls arrive.
"""
import os
import numpy as np
import ml_dtypes
from contextlib import ExitStack

import concourse.bacc as bacc
import concourse.tile as tile
import concourse.mybir as mybir
from concourse.bass import AP
from concourse.bass_utils import run_bass_kernel_spmd

F32 = mybir.dt.float32
F16 = mybir.dt.float16
BF16 = mybir.dt.bfloat16
F8 = mybir.dt.float8e4
AF = mybir.ActivationFunctionType
OP = mybir.AluOpType

B, T, D = 4, 4096, 1024
DL = 512          # D-half per core
TC = 512          # time chunk
NCORES = 8

_NC_CACHE = {}


def _ap_sig(pap):
    return (str(pap.memref), pap.offset, tuple(map(tuple, pap.ap)), pap.dtype)


def _dedupe_ldweights(nc):
    """Delete an InstLdweights whose weights AP matches the previous
    InstLdweights in the same block's PE stream, when it carries no sync
    info. The PE array then just keeps the already-loaded weights."""
    removed = 0
    for f in nc.m.functions:
        for blk in f.blocks:
            insts = list(blk.instructions)
            last_sig = None
            keep = []
            for i in insts:
                nm = type(i).__name__
                if nm == 'InstLdweights':
                    sig = (_ap_sig(i.ins[0]), i.perf_mode, i.is_transpose,
                           i.tile_position, i.tile_size)
                    si = i.sync_info
                    clean = si is None or (len(si.on_wait) == 0
                                           and len(si.on_update) == 0)
                    if sig == last_sig and clean:
                        removed += 1
                        continue
                    last_sig = sig
                elif nm == 'InstMatmult':
                    pass          # matmuls don't invalidate loaded weights
                elif getattr(i, 'engine', None) == mybir.EngineType.PE:
                    last_sig = None  # drains/branches on PE: be conservative
                keep.append(i)
            if removed and len(keep) != len(insts):
                blk.instructions.clear()
                for i in keep:
                    blk.instructions.append(i)
    return removed


def _build(D_=D, DL_=DL, T_=T, TC_=TC, n_devices=NCORES):
    KB, MB, NCH = D_ // 128, DL_ // 128, T_ // TC_
    YW = KB * TC_          # per-chunk packed y width (4096)
    NP = KB // 2           # fp8 DoubleRow kb-pairs

    nc = bacc.Bacc("TRN2", target_bir_lowering=False, debug=False,
                   num_devices=n_devices)
    # all inputs pre-blocked on the host to [128, ...] partition-major
    yk = nc.dram_tensor("yk", (128, NCH * YW), BF16, kind="ExternalInput").ap()
    yv = nc.dram_tensor("yv", (128, NCH * YW), BF16, kind="ExternalInput").ap()
    yr = nc.dram_tensor("yr", (128, NCH * YW), F8, kind="ExternalInput").ap()
    wk = nc.dram_tensor("wk", (128, KB * DL_), BF16, kind="ExternalInput").ap()
    wv = nc.dram_tensor("wv", (128, KB * DL_), BF16, kind="ExternalInput").ap()
    wr = nc.dram_tensor("wr", (128, KB * DL_), F8, kind="ExternalInput").ap()
    wo = nc.dram_tensor("wo", (128, MB * D_), F16, kind="ExternalInput").ap()
    cvec = nc.dram_tensor("cvec", (128, MB), F16, kind="ExternalInput").ap()
    ewv = nc.dram_tensor("ewv", (128, MB), F16, kind="ExternalInput").ap()
    out = nc.dram_tensor("out", (T_, D_), F16, kind="ExternalOutput").ap()

    with tile.TileContext(nc) as tc, ExitStack() as ctx:
        wpool = ctx.enter_context(tc.tile_pool(name="weights", bufs=1))

        # PE warmup: the warm tile is memset on the DVE (ready ~2us; the
        # gpsimd DGE spends ~8us in its library-load preamble) so the dummy
        # matmuls keep the PE HAM clock at 2.4 GHz until real work arrives.
        warm = wpool.tile([128, 640], BF16, tag="warm")
        nc.vector.memset(warm[:], 0.0)

        # Head loads in kb-pair pieces, spread over three DGE queues so the
        # chunk-0 projections can start on piece 0 while later pieces are
        # still in flight (dep tracking is range-granular):
        #   sync DGE:   y0k/wk interleaved, then cvec/ew (k-phase critical)
        #   scalar DGE: y0v/wv interleaved, then wo
        #   gpsimd DGE: y0r/wr, then the chunk-1+ loads follow in the s-loop
        y0k = wpool.tile([128, YW], BF16, tag="y0k")
        wkall = wpool.tile([128, KB * DL_], BF16, tag="wk")
        for p in range(NP):
            nc.sync.dma_start(y0k[:, p * 2 * TC_:(p + 1) * 2 * TC_],
                              yk[:, p * 2 * TC_:(p + 1) * 2 * TC_])
            nc.sync.dma_start(wkall[:, p * 2 * DL_:(p + 1) * 2 * DL_],
                              wk[:, p * 2 * DL_:(p + 1) * 2 * DL_])
        cvec_sb = wpool.tile([128, MB], F16, tag="cvec")
        nc.sync.dma_start(cvec_sb[:], cvec[:])
        ew_dma = wpool.tile([128, MB], F16, tag="ew_dma")
        nc.sync.dma_start(ew_dma[:], ewv[:])
        # copy on the DVE so the stride-0 broadcast reads used by the scans
        # are ordered behind the write by same-engine program order
        ew_sb = wpool.tile([128, MB], F16, tag="ew_sb")
        nc.vector.tensor_copy(ew_sb[:], ew_dma[:])

        y0v = wpool.tile([128, YW], BF16, tag="y0v")
        wvall = wpool.tile([128, KB * DL_], BF16, tag="wv")
        for p in range(NP):
            nc.scalar.dma_start(y0v[:, p * 2 * TC_:(p + 1) * 2 * TC_],
                                yv[:, p * 2 * TC_:(p + 1) * 2 * TC_])
            nc.scalar.dma_start(wvall[:, p * 2 * DL_:(p + 1) * 2 * DL_],
                                wv[:, p * 2 * DL_:(p + 1) * 2 * DL_])
        woall = wpool.tile([128, MB * D_], F16, tag="wo")
        nc.scalar.dma_start(woall[:], wo[:])

        y0r = []
        wr_sb = []
        for p in range(NP):
            t = wpool.tile([128, 2, TC_], F8, tag=f"y0r{p}")
            nc.gpsimd.dma_start(t[:], yr[:, p * 2 * TC_:(p + 1) * 2 * TC_])
            y0r.append(t)
            t = wpool.tile([128, 2, DL_], F8, tag=f"wr{p}")
            nc.gpsimd.dma_start(t[:], wr[:, p * 2 * DL_:(p + 1) * 2 * DL_])
            wr_sb.append(t)

        y_pool = ctx.enter_context(tc.tile_pool(name="y", bufs=2))
        pp_pool = ctx.enter_context(tc.tile_pool(name="pp", bufs=4, space="PSUM"))
        po_pool = ctx.enter_context(tc.tile_pool(name="po", bufs=1, space="PSUM"))
        ee_pool = ctx.enter_context(tc.tile_pool(name="ee", bufs=2))
        er_pool = ctx.enter_context(tc.tile_pool(name="er", bufs=2))
        ab_pool = ctx.enter_context(tc.tile_pool(name="ab", bufs=2))
        nd_pool = ctx.enter_context(tc.tile_pool(name="nd", bufs=2))
        dn_pool = ctx.enter_context(tc.tile_pool(name="dn", bufs=2))
        vv_pool = ctx.enter_context(tc.tile_pool(name="vv", bufs=2))
        ws_pool = ctx.enter_context(tc.tile_pool(name="ws", bufs=2))
        ob_pool = ctx.enter_context(tc.tile_pool(name="ob", bufs=2))

        def ew_bc(mb, ln):
            base = ew_sb[:, mb:mb + 1]
            return AP(base.tensor, base.offset, [list(base.ap[0]), [0, ln]])

        prevA = [None] * MB
        prevB = [None] * MB
        # segments: (chunk, off, ln, load). The last chunk is split in half
        # so its out-projection overlaps the tail of the vector chain.
        segs = [(c, 0, TC_, True) for c in range(NCH - 1)]
        segs += [(NCH - 1, 0, TC_ // 2, True),
                 (NCH - 1, TC_ // 2, TC_ // 2, False)]
        wss_hist = {}
        ykt = yvt = yrt = None
        for s in range(len(segs) + 1):
            if s < len(segs):
                c, off, ln, load = segs[s]
                if load:
                    if c == 0:
                        ykt, yvt, yrt = y0k, y0v, y0r
                    else:
                        # spread the per-chunk loads over two DGE queues:
                        # per-queue transfers are FIFO at ~140GB/s, and
                        # 2.5MB/chunk on one queue leaves no slack against
                        # the 25us chunk cadence
                        ykt = y_pool.tile([128, YW], BF16, tag="yk")
                        nc.gpsimd.dma_start(ykt[:], yk[:, c * YW:(c + 1) * YW])
                        yvt = y_pool.tile([128, YW], BF16, tag="yv")
                        nc.scalar.dma_start(yvt[:], yv[:, c * YW:(c + 1) * YW])
                        yrt = []
                        for p in range(NP):
                            yt = y_pool.tile([128, 2, TC_], F8, tag=f"yr{p}")
                            nc.gpsimd.dma_start(
                                yt[:],
                                yr[:, c * YW + p * 2 * TC_:
                                   c * YW + (p + 1) * 2 * TC_])
                            yrt.append(yt)

                wss = []
                ees, ers, vvs = [], [], []
                if s == 0:
                    # chunk 0: kb-outer phased projections so the first
                    # matmuls only need DMA piece 0, not the whole 1MB
                    # weight + 1MB y load (pieces land progressively).
                    kps = [pp_pool.tile([128, TC_], F32, tag="pp",
                                        name=f"kp0_{mb}") for mb in range(MB)]
                    for i in range(8):
                        nc.tensor.matmul(kps[0][:], warm[:, 0:128],
                                         warm[:, 128:640],
                                         start=True, stop=True)
                    for kb in range(KB):
                        for mb in range(MB):
                            nc.tensor.matmul(
                                kps[mb][:],
                                wkall[:, kb * DL_ + mb * 128:
                                      kb * DL_ + (mb + 1) * 128],
                                ykt[:, kb * TC_:kb * TC_ + TC_],
                                start=(kb == 0), stop=(kb == KB - 1))
                    for mb in range(MB):
                        ee = ee_pool.tile([128, 2 * TC_], F16, tag=f"ee{mb}",
                                          name=f"ee0_{mb}")
                        nc.scalar.activation(ee[:, TC_:2 * TC_], kps[mb][:],
                                             AF.Exp)
                        ees.append(ee)
                    vps = [pp_pool.tile([128, TC_], F32, tag="pp",
                                        name=f"vp0_{mb}") for mb in range(MB)]
                    for kb in range(KB):
                        for mb in range(MB):
                            nc.tensor.matmul(
                                vps[mb][:],
                                wvall[:, kb * DL_ + mb * 128:
                                      kb * DL_ + (mb + 1) * 128],
                                yvt[:, kb * TC_:kb * TC_ + TC_],
                                start=(kb == 0), stop=(kb == KB - 1))
                    for mb in range(MB):
                        vv = vv_pool.tile([128, TC_], F16, tag=f"vv{mb}",
                                          name=f"vv0_{mb}")
                        nc.scalar.copy(vv[:], vps[mb][:])
                        vvs.append(vv)
                    rps = [pp_pool.tile([128, TC_], F32, tag="pp",
                                        name=f"rp0_{mb}") for mb in range(MB)]
                    for p in range(NP):
                        for mb in range(MB):
                            nc.tensor.matmul(
                                rps[mb][:],
                                wr_sb[p][:, :, mb * 128:(mb + 1) * 128],
                                yrt[p][:, :, 0:TC_],
                                start=(p == 0), stop=(p == NP - 1),
                                perf_mode=mybir.MatmulPerfMode.DoubleRow)
                    for mb in range(MB):
                        er = er_pool.tile([128, TC_], F32, tag=f"er{mb}",
                                          name=f"er0_{mb}")
                        nc.scalar.activation(er[:], rps[mb][:], AF.Exp,
                                             scale=-1.0)
                        ers.append(er)
                for mb in range(MB):
                    if s == 0:
                        ee, er, vv = ees[mb], ers[mb], vvs[mb]
                    else:
                        kp = pp_pool.tile([128, TC_], F32, tag="pp")
                        for kb in range(KB):
                            nc.tensor.matmul(
                                kp[:, 0:ln],
                                wkall[:, kb * DL_ + mb * 128:
                                      kb * DL_ + (mb + 1) * 128],
                                ykt[:, kb * TC_ + off:kb * TC_ + off + ln],
                                start=(kb == 0), stop=(kb == KB - 1))
                        vp = pp_pool.tile([128, TC_], F32, tag="pp")
                        for kb in range(KB):
                            nc.tensor.matmul(
                                vp[:, 0:ln],
                                wvall[:, kb * DL_ + mb * 128:
                                      kb * DL_ + (mb + 1) * 128],
                                yvt[:, kb * TC_ + off:kb * TC_ + off + ln],
                                start=(kb == 0), stop=(kb == KB - 1))
                        rp = pp_pool.tile([128, TC_], F32, tag="pp")
                        for p in range(NP):
                            nc.tensor.matmul(
                                rp[:, 0:ln],
                                wr_sb[p][:, :, mb * 128:(mb + 1) * 128],
                                yrt[p][:, :, off:off + ln],
                                start=(p == 0), stop=(p == NP - 1),
                                perf_mode=mybir.MatmulPerfMode.DoubleRow)

                        # EE = [ekv | ek] f16
                        ee = ee_pool.tile([128, 2 * TC_], F16, tag=f"ee{mb}")
                        nc.scalar.activation(ee[:, TC_:TC_ + ln], kp[:, 0:ln],
                                             AF.Exp)
                        # er = exp(-r) f32 (f32 so large -r can't overflow)
                        er = er_pool.tile([128, TC_], F32, tag=f"er{mb}")
                        nc.scalar.activation(er[:, 0:ln], rp[:, 0:ln], AF.Exp,
                                             scale=-1.0)
                        # evict v via the scalar engine: frees the PSUM bank
                        # for the PE and makes the ekv multiply all-f16
                        vv = vv_pool.tile([128, TC_], F16, tag=f"vv{mb}")
                        nc.scalar.copy(vv[:, 0:ln], vp[:, 0:ln])
                    nc.vector.tensor_tensor(ee[:, 0:ln], ee[:, TC_:TC_ + ln],
                                            vv[:, 0:ln], OP.mult)

                    # A/B scan outputs in f16, decay as stride-0 broadcast
                    ab = ab_pool.tile([128, 2 * TC_], F16, tag=f"ab{mb}")
                    initA = 0.0 if s == 0 else prevA[mb]
                    nc.vector.tensor_tensor_scan(ab[:, 0:ln], ew_bc(mb, ln),
                                                 ee[:, 0:ln], initA,
                                                 OP.mult, OP.add)
                    initB = 0.0 if s == 0 else prevB[mb]
                    nc.vector.tensor_tensor_scan(ab[:, TC_:TC_ + ln],
                                                 ew_bc(mb, ln),
                                                 ee[:, TC_:TC_ + ln], initB,
                                                 OP.mult, OP.add)
                    prevA[mb] = ab[:, ln - 1:ln]
                    prevB[mb] = ab[:, TC_ + ln - 1:TC_ + ln]

                    # [nm | dn] = c*[ekv | ek] + [A | B] in ONE wide STT
                    cs = cvec_sb[:, mb:mb + 1]
                    nd = nd_pool.tile([128, 2 * TC_], F16, tag=f"nd{mb}")
                    if ln == TC_:
                        nc.vector.scalar_tensor_tensor(nd[:], ee[:], cs,
                                                       ab[:], OP.mult, OP.add)
                    else:
                        nc.vector.scalar_tensor_tensor(
                            nd[:, 0:ln], ee[:, 0:ln], cs, ab[:, 0:ln],
                            OP.mult, OP.add)
                        nc.vector.scalar_tensor_tensor(
                            nd[:, TC_:TC_ + ln], ee[:, TC_:TC_ + ln], cs,
                            ab[:, TC_:TC_ + ln], OP.mult, OP.add)

                    # dn' = (er + 1) * dn ; ws = nm / dn'
                    dnp = dn_pool.tile([128, TC_], F32, tag=f"dn{mb}")
                    nc.vector.scalar_tensor_tensor(dnp[:, 0:ln], er[:, 0:ln],
                                                   1.0, nd[:, TC_:TC_ + ln],
                                                   OP.add, OP.mult)
                    nc.vector.reciprocal_approx_fast(dnp[:, 0:ln],
                                                     dnp[:, 0:ln])
                    ws = ws_pool.tile([128, TC_], F16, tag=f"ws{mb}")
                    nc.vector.tensor_tensor(ws[:, 0:ln], nd[:, 0:ln],
                                            dnp[:, 0:ln], OP.mult)
                    wss.append(ws)
                wss_hist[s] = (wss, c * TC_ + off, ln)

            if s >= 1:
                # out-projection for segment s-1 (deferred so the PE never
                # waits on the vector chain of the same segment)
                wssp, tbase, lnp = wss_hist.pop(s - 1)
                for pair in range(lnp // 256):
                    pos = [po_pool.tile([128, D_], F32, tag=f"po{i}",
                                        name=f"po{i}") for i in range(2)]
                    for mb in range(MB):
                        for i, tb in enumerate((pair * 2, pair * 2 + 1)):
                            for half in range(2):
                                nc.tensor.matmul(
                                    pos[i][:, half * 512:(half + 1) * 512],
                                    wssp[mb][:, tb * 128:(tb + 1) * 128],
                                    woall[:, mb * D_ + half * 512:
                                          mb * D_ + (half + 1) * 512],
                                    start=(mb == 0), stop=(mb == MB - 1))
                    for i, tb in enumerate((pair * 2, pair * 2 + 1)):
                        ob = ob_pool.tile([128, D_], F16, tag="ob")
                        nc.scalar.copy(ob[:], pos[i][:])
                        nc.sync.dma_start(
                            out[tbase + tb * 128:tbase + (tb + 1) * 128, :],
                            ob[:])

    nc.compile()
    n = _dedupe_ldweights(nc)
    if os.environ.get("KERNEL_DEBUG"):
        print(f"dedupe_ldweights removed {n}")
    return nc


def get_nc():
    if "nc" not in _NC_CACHE:
        _NC_CACHE["nc"] = _build()
    return _NC_CACHE["nc"]


def _blk_y(arr, dt):
    # [D, T] -> [128, NCH*KB*TC]: y[kb*128+p, c*TC+t] -> [p, c*YW + kb*TC + t]
    KB, NCH = D // 128, T // TC
    a = arr.reshape(KB, 128, NCH, TC).transpose(1, 2, 0, 3)
    return np.ascontiguousarray(a.reshape(128, NCH * KB * TC)).astype(dt)


def _blk_yr(arr, dt):
    # [D, T] -> [128, NCH*KB*TC] with DoubleRow kb-pair packing:
    # yr[(2q+i)*128+p, c*TC+t] -> [p, c*YW + q*2*TC + i*TC + t]
    KB, NCH = D // 128, T // TC
    a = arr.reshape(KB // 2, 2, 128, NCH, TC).transpose(2, 3, 0, 1, 4)
    return np.ascontiguousarray(a.reshape(128, NCH * KB * TC)).astype(dt)


def _blk_w(arr, dt):
    # [D, DL] -> [128, KB*DL]: w[kb*128+p, j] -> [p, kb*DL + j]
    KB = D // 128
    a = arr.reshape(KB, 128, DL).transpose(1, 0, 2)
    return np.ascontiguousarray(a.reshape(128, KB * DL)).astype(dt)


def _blk_wr(arr, dt):
    # [D, DL] -> [128, KB*DL] with DoubleRow kb-pair packing
    KB = D // 128
    a = arr.reshape(KB // 2, 2, 128, DL).transpose(2, 0, 1, 3)
    return np.ascontiguousarray(a.reshape(128, KB * DL)).astype(dt)


def _blk_wo(arr, dt):
    # [DL, D] -> [128, MB*D]: wo[mb*128+p, i] -> [p, mb*D + i]
    MB = DL // 128
    a = arr.reshape(MB, 128, D).transpose(1, 0, 2)
    return np.ascontiguousarray(a.reshape(128, MB * D)).astype(dt)


def make_in_maps(x, time_decay, time_first, time_mix_k, time_mix_v, time_mix_r,
                 W_key, W_value, W_receptance, W_output):
    x = np.asarray(x, np.float32)
    time_decay = np.asarray(time_decay, np.float64)
    time_first = np.asarray(time_first, np.float64)
    mk = np.asarray(time_mix_k, np.float32).reshape(-1)
    mv = np.asarray(time_mix_v, np.float32).reshape(-1)
    mr = np.asarray(time_mix_r, np.float32).reshape(-1)
    W_key = np.asarray(W_key, np.float32)
    W_value = np.asarray(W_value, np.float32)
    W_receptance = np.asarray(W_receptance, np.float32)
    W_output = np.asarray(W_output, np.float32)

    MB = DL // 128
    ew = np.exp(-np.exp(time_decay))
    c = (ew * np.exp(time_first) - 1.0).astype(np.float32)
    ew = ew.astype(np.float32)

    def blocked(vec, nb, dt):
        return np.ascontiguousarray(vec.reshape(nb, 128).T.astype(dt))

    # host time-mix: y_p[b] = (x*m_p + last_x*(1-m_p)).T  as [D, T]
    last_x = np.concatenate([np.zeros((B, 1, D), np.float32), x[:, :-1, :]],
                            axis=1)
    ys = {}
    for nm, m in (("yk", mk), ("yv", mv), ("yr", mr)):
        ym = x * m + last_x * (1.0 - m)
        if nm == "yr":
            ys[nm] = [_blk_yr(ym[b].T, ml_dtypes.float8_e4m3)
                      for b in range(B)]
        else:
            ys[nm] = [_blk_y(ym[b].T, ml_dtypes.bfloat16) for b in range(B)]

    halves = []
    for h in range(2):
        dsl = slice(h * DL, (h + 1) * DL)
        halves.append({
            "wk": _blk_w(W_key.T[:, dsl], ml_dtypes.bfloat16),
            "wv": _blk_w(W_value.T[:, dsl], ml_dtypes.bfloat16),
            "wr": _blk_wr(W_receptance.T[:, dsl], ml_dtypes.float8_e4m3),
            "wo": _blk_wo(W_output.T[dsl, :], np.float16),
            "cvec": blocked(c[dsl], MB, np.float16),
            "ewv": blocked(ew[dsl], MB, np.float16),
        })

    in_maps = []
    for i in range(NCORES):
        b, h = i // 2, i % 2
        m = dict(halves[h])
        m["yk"] = ys["yk"][b]
        m["yv"] = ys["yv"][b]
        m["yr"] = ys["yr"][b]
        in_maps.append(m)
    return in_maps


def run(in_maps, trace=False):
    nc = get_nc()
    return run_bass_kernel_spmd(nc, in_maps, core_ids=list(range(NCORES)),
                                trace=trace)


def kernel(**inputs):
    in_maps = make_in_maps(**inputs)
    res = run(in_maps, trace=bool(int(os.environ.get("KERNEL_TRACE", "0"))))
    out = np.zeros((B, T, D), np.float32)
    for i in range(NCORES):
        out[i // 2] += res.results[i]["out"].astype(np.float32)
    if res.exec_time_ns is not None:
        print(f"HW exec time: {res.exec_time_ns} ns")
    return out
